# revision 1
# baseline (speedup 1.0000x reference)
"""Bass/Trainium2 kernel for nn_BottomUpHTMM (bottom-up hidden tree Markov model).

Tree: complete 4-ary, depth 7, 21845 nodes. N_GEN=16 generative models, C=8
states, 256 labels.

Sharding: the 16 depth-5 subtrees rooted at level-2 nodes (5..20) are split
2-per-core across 8 cores. All 16 gens stay on every core so the partition dim
is fully used: (g, c) = 16*8 = 128 partitions, nodes along the free dim.

Per level, the upward/downward einsums contract (child state j, child slot l)
via 4 PSUM-accumulating matmuls with block-diagonal [128,128] weights
W_l[(g,j),(g',i)] = delta_gg' * a_sp[g,i,j,l], reading child columns with a
stride-4 AP (children of parent p are exactly local nodes 4p..4p+3).
Normalization over states (partitions within a g-block) uses a matmul with
block-ones. Emissions sm_b[:, labels] come from one ap_gather per core.
Tree top (levels 0-1, 5 nodes) is computed redundantly on every core after a
2KB AllGather of the 16 subtree-root (prior, beta) columns.

The 4 log-lik terms are reduced on device into 62 partial columns [128, 64]
(per (g,i) row); the host sums rows/cores and applies log_softmax(sp) weights.
"""

import numpy as np

L = 4
DEPTH = 7
G = 16
C = 8
M = 256
NCORES = 8
T_SIZE = 21845
LIM = [0, 1, 5, 21, 85, 341, 1365, 5461, 21845]

# per-core local levels: j=0..5 <-> global levels 2..7
P_LVL = [2, 8, 32, 128, 512, 2048]           # nodes per local level per core
OFF_LVL = [0, 2, 10, 42, 170, 682]           # col offset of each level in per-core node list
N_SUB = 2730                                  # per-core nodes
N_SUB_PAD = 2736                              # padded to %16 for ap_gather
UP_CHUNK = 256                                # parents per upward matmul chunk
DN_CHUNK = 128                                # parents per downward matmul chunk

# ---- output partial column map (out_part [128, 64]) ----
COL_A_TOP0 = 0
COL_A_TOP1 = 1
COL_RHO_TOP0 = 2      # 2..5   l=0..3
COL_RHO_TOP1 = 6      # 6..9
COL_B_NODE0 = 10
COL_B_N14 = 11
COL_B_ROOTS = 12      # counted from core 0 only
COL_PI = 13
COL_A_SUB = 14        # 14..21   8 chunk-units
COL_RHO_SUB = 22      # 22..53   8 chunk-units x 4 l
COL_B_SUB = 54        # 54..61   8 chunk-units
NCOLS = 64

# downward chunk-units: (level j, p0, p1)
DN_UNITS = []
for _j in range(5):
    _P = P_LVL[_j]
    for _p0 in range(0, _P, DN_CHUNK):
        DN_UNITS.append((_j, _p0, min(_p0 + DN_CHUNK, _P)))
assert len(DN_UNITS) == 8


def _wrap_idx(idx, pad_to):
    """Pack index list for ap_gather: idx j at partition j%16, slot j//16,
    replicated across the 8 gpsimd cores (16-partition groups)."""
    idx = np.asarray(idx, dtype=np.uint16)
    n = len(idx)
    assert pad_to % 16 == 0 and n <= pad_to
    full = np.zeros(pad_to, dtype=np.int16)
    full[:n] = idx
    grid = full.reshape(pad_to // 16, 16).T  # [16, pad/16]
    return np.tile(grid, (8, 1))             # [128, pad/16]


def _host_prep(t):
    """Per-core node lists, labels, gather indices. Verifies tree layout."""
    t = np.asarray(t)
    labels = t[:, 0].astype(np.int64)
    assert t.shape == (T_SIZE, 7)
    # children of node n are 4n+1..4n+4 -> per-core local children at 4p+l
    cores = []
    for k in range(NCORES):
        roots = [5 + 2 * k, 6 + 2 * k]
        nodes = []
        for j in range(6):
            sz = 4 ** j
            for r in roots:
                start = sz * r + (sz - 1) // 3
                nodes.append(np.arange(start, start + sz))
        nodes = np.concatenate(nodes)
        assert nodes.shape == (N_SUB,)
        lab = labels[nodes]
        # leaves pos parity check: pos = (n-1)%4 == local_leaf_index % 4
        leaf_nodes = nodes[OFF_LVL[5]:]
        assert np.array_equal((leaf_nodes - 1) % 4, np.arange(2048) % 4)
        cores.append({
            "idx_sub": _wrap_idx(lab, N_SUB_PAD),
            "idx_sel": _wrap_idx([2 * k, 2 * k + 1] + [0] * 14, 16),
        })
    idx_top = _wrap_idx(labels[:21], 32)
    return cores, idx_top


def build_bass(debug=False, max_phase=99, sub=99):
    import concourse.bacc as bacc
    import concourse.tile as tile
    import concourse.mybir as mybir
    from concourse import bass

    f32 = mybir.dt.float32
    i16 = mybir.dt.uint16
    AF = mybir.ActivationFunctionType
    ALU = mybir.AluOpType
    AX = mybir.AxisListType

    nc = bacc.Bacc("TRN2", target_bir_lowering=False, debug=False,
                   num_devices=NCORES)

    # ---- I/O ----
    a_in = nc.dram_tensor("a_perm", [128, 32], f32, kind="ExternalInput").ap()
    b_in = nc.dram_tensor("b_gc", [128, 256], f32, kind="ExternalInput").ap()
    pi_in = nc.dram_tensor("pi_gc", [128, 4], f32, kind="ExternalInput").ap()
    sp_in = nc.dram_tensor("sp_bc", [128, 4], f32, kind="ExternalInput").ap()
    mbd_in = nc.dram_tensor("mbd", [128, 128], f32, kind="ExternalInput").ap()
    ixs_in = nc.dram_tensor("idx_sub", [128, N_SUB_PAD // 16], i16, kind="ExternalInput").ap()
    ixt_in = nc.dram_tensor("idx_top", [128, 2], i16, kind="ExternalInput").ap()
    ixl_in = nc.dram_tensor("idx_sel", [128, 1], i16, kind="ExternalInput").ap()
    o_part = nc.dram_tensor("out_part", [128, NCOLS], f32, kind="ExternalOutput").ap()
    dbg_out = {}

    def dbg(name, shape):
        if debug:
            dbg_out[name] = nc.dram_tensor("dbg_" + name, shape, f32,
                                           kind="ExternalOutput").ap()
            return dbg_out[name]
        return None

    class _PhaseDoneT(Exception):
        pass
    _PhaseDone = _PhaseDoneT()
    with tile.TileContext(nc) as tc:
        with tc.tile_pool(name="per", bufs=1) as per, \
             tc.tile_pool(name="wrk", bufs=3) as wrk, \
             tc.tile_pool(name="ps", bufs=2, space="PSUM") as ps, \
             tc.tile_pool(name="dram", bufs=1, space="DRAM") as dram:

            def _emit():

              # ---------- load inputs ----------
              at = per.tile([128, 32], f32, tag="at")          # a_perm [(g,j),(i,l)]
              bt = per.tile([128, 256], f32, tag="bt")
              pit = per.tile([128, 4], f32, tag="pit")
              spt = per.tile([128, 4], f32, tag="spt")
              mbd = per.tile([128, 128], f32, tag="mbd")
              ixs = per.tile([128, N_SUB_PAD // 16], i16, tag="ixs")
              ixt = per.tile([128, 2], i16, tag="ixt")
              ixl = per.tile([128, 1], i16, tag="ixl")
              for dst, src in [(at, a_in), (bt, b_in), (pit, pi_in), (spt, sp_in),
                               (mbd, mbd_in), (ixs, ixs_in), (ixt, ixt_in), (ixl, ixl_in)]:
                  nc.sync.dma_start(out=dst[:], in_=src)

              out_part = per.tile([128, NCOLS], f32, tag="out_part")
              nc.vector.memset(out_part[:], 0.0)

              # ---------- softmaxes (no max-subtraction; inputs are ~N(0,1)) ----------
              # sm_b over labels (free)
              sm_b = per.tile([128, 256], f32, tag="sm_b")
              log_b = per.tile([128, 256], f32, tag="log_b")
              s1 = wrk.tile([128, 1], f32, tag="s1")
              nc.scalar.activation(out=sm_b[:], in_=bt[:], func=AF.Exp, accum_out=s1[:])
              r1 = wrk.tile([128, 1], f32, tag="s1")
              nc.vector.reciprocal(r1[:], s1[:])
              nc.vector.tensor_scalar_mul(sm_b[:], sm_b[:], r1[:])
              nc.scalar.activation(out=log_b[:], in_=sm_b[:], func=AF.Ln)

              # sm_sp over l (free); rows (g,*) identical
              sm_sp = per.tile([128, 4], f32, tag="sm_sp")
              s2 = wrk.tile([128, 1], f32, tag="s1")
              nc.scalar.activation(out=sm_sp[:], in_=spt[:], func=AF.Exp, accum_out=s2[:])
              r2 = wrk.tile([128, 1], f32, tag="s1")
              nc.vector.reciprocal(r2[:], s2[:])
              nc.vector.tensor_scalar_mul(sm_sp[:], sm_sp[:], r2[:])

              # sm_pi over c (partitions within g-block): exp -> block-ones matmul -> recip -> mul
              sm_pi = per.tile([128, 4], f32, tag="sm_pi")
              log_pi = per.tile([128, 4], f32, tag="log_pi")
              pie = wrk.tile([128, 4], f32, tag="pie")
              nc.scalar.activation(out=pie[:], in_=pit[:], func=AF.Exp)
              ps_pi = ps.tile([128, 4], f32, tag="cs")
              nc.tensor.matmul(ps_pi[:], mbd[:], pie[:], start=True, stop=True)
              rpi = wrk.tile([128, 4], f32, tag="pie")
              nc.vector.reciprocal(rpi[:], ps_pi[:])
              nc.vector.tensor_tensor(sm_pi[:], pie[:], rpi[:], ALU.mult)
              nc.scalar.activation(out=log_pi[:], in_=sm_pi[:], func=AF.Ln)

              # sm_a over i (free, stride 4 in (i,l) layout)
              ae = wrk.tile([128, 32], f32, tag="ae")
              nc.scalar.activation(out=ae[:], in_=at[:], func=AF.Exp)
              sa = wrk.tile([128, 4], f32, tag="pie")
              ae_li = ae[:].rearrange("p (i l) -> p l i", l=4)
              nc.vector.tensor_reduce(sa[:], ae_li, axis=AX.X, op=ALU.add)
              ra = wrk.tile([128, 4], f32, tag="pie")
              nc.vector.reciprocal(ra[:], sa[:])
              sm_a = per.tile([128, 32], f32, tag="sm_a")
              nc.vector.tensor_tensor(
                  sm_a[:].rearrange("p (i l) -> p l i", l=4), ae_li,
                  ra[:, :, None].to_broadcast([128, 4, 8]), ALU.mult)
              log_a = per.tile([128, 32], f32, tag="log_a")
              nc.scalar.activation(out=log_a[:], in_=sm_a[:], func=AF.Ln)
              # asp = sm_a * sm_sp[l];  v8 = asp * log_a
              asp = per.tile([128, 32], f32, tag="asp")
              nc.vector.tensor_tensor(
                  asp[:].rearrange("p (i l) -> p i l", l=4),
                  sm_a[:].rearrange("p (i l) -> p i l", l=4),
                  sm_sp[:][:, None, :].to_broadcast([128, 8, 4]), ALU.mult)
              v8 = per.tile([128, 32], f32, tag="v8")
              nc.vector.tensor_tensor(v8[:], asp[:], log_a[:], ALU.mult)

              # weights W_l, V_l [128, 128]: Mbd * bcast of (i)-column l
              # W_l[(g,j), (g',i)] = mbd * asp[(g,j), i*4+l]
              W = []
              V = []
              for l in range(L):
                  w_l = per.tile([128, 128], f32, tag=f"w{l}")
                  asp_l = asp[:].rearrange("p (i l) -> p i l", l=4)[:, :, l]
                  nc.vector.tensor_tensor(
                      w_l[:].rearrange("p (a b) -> p a b", a=16),
                      mbd[:].rearrange("p (a b) -> p a b", a=16),
                      asp_l[:, None, :].to_broadcast([128, 16, 8]), ALU.mult)
                  W.append(w_l)
                  v_l = per.tile([128, 128], f32, tag=f"v{l}")
                  v8_l = v8[:].rearrange("p (i l) -> p i l", l=4)[:, :, l]
                  nc.vector.tensor_tensor(
                      v_l[:].rearrange("p (a b) -> p a b", a=16),
                      mbd[:].rearrange("p (a b) -> p a b", a=16),
                      v8_l[:, None, :].to_broadcast([128, 16, 8]), ALU.mult)
                  V.append(v_l)

              # ---------- emission gathers ----------
              if max_phase < 1:
                  nc.sync.dma_start(out=o_part, in_=out_part[:])
                  return
              emis = per.tile([128, N_SUB_PAD], f32, tag="emis")
              logb_s = per.tile([128, N_SUB_PAD], f32, tag="logb_s")
              for c0, c1 in [(0, 1024), (1024, 2048), (2048, N_SUB_PAD)]:
                  nc.gpsimd.indirect_copy(emis[:, c0:c1], sm_b[:],
                                          ixs[:, c0 // 16:c1 // 16], True)
                  nc.gpsimd.indirect_copy(logb_s[:, c0:c1], log_b[:],
                                          ixs[:, c0 // 16:c1 // 16], True)
              emis_t = per.tile([128, 32], f32, tag="emis_t")
              logb_t = per.tile([128, 32], f32, tag="logb_t")
              nc.gpsimd.indirect_copy(emis_t[:], sm_b[:], ixt[:], True)
              nc.gpsimd.indirect_copy(logb_t[:], log_b[:], ixt[:], True)

              # ---------- per-level storage ----------
              pb = [per.tile([128, 2, P_LVL[j]], f32, tag=f"pb{j}", name=f"pb{j}") for j in range(6)]
              bnr = [per.tile([128, P_LVL[j]], f32, tag=f"bnr{j}", name=f"bnr{j}") for j in range(5)]
              eps = [per.tile([128, P_LVL[j]], f32, tag=f"eps{j}", name=f"eps{j}") for j in range(6)]

              # ---------- leaves (local level 5) ----------
              if max_phase < 2:
                  nc.sync.dma_start(out=o_part, in_=out_part[:])
                  return
              pi_per = sm_pi[:][:, None, :].to_broadcast([128, 512, 4])
              bun = per.tile([128, 2048], f32, tag="bun")
              nc.vector.tensor_tensor(
                  bun[:].rearrange("p (r l) -> p r l", l=4),
                  emis[:, OFF_LVL[5]:N_SUB].rearrange("p (r l) -> p r l", l=4),
                  pi_per, ALU.mult)
              nc.vector.tensor_copy(
                  out=pb[5][:, 0, :].rearrange("p (r l) -> p r l", l=4), in_=pi_per)
              for c in range(4):
                  sl = slice(512 * c, 512 * (c + 1))
                  ps_n = ps.tile([128, 512], f32, tag="cs")
                  nc.tensor.matmul(ps_n[:], mbd[:], bun[:, sl], start=True, stop=True)
                  rn = wrk.tile([128, 512], f32, tag="rn")
                  nc.vector.reciprocal(rn[:], ps_n[:])
                  nc.vector.tensor_tensor(pb[5][:, 1, sl], bun[:, sl], rn[:], ALU.mult)

              # ---------- upward: local levels j=4..0 ----------
              if max_phase < 3:
                  nc.sync.dma_start(out=o_part, in_=out_part[:])
                  return
              up_units = []
              for j in range(4, -1, -1):
                  P = P_LVL[j]
                  for p0 in range(0, P, UP_CHUNK):
                      up_units.append((j, p0, min(p0 + UP_CHUNK, P)))

              for (j, p0, p1) in up_units:
                  Pc = p1 - p0
                  child = pb[j + 1][:].rearrange("p q (n l) -> p q l n", l=4)
                  ps_ub = ps.tile([128, 2, Pc], f32, tag="ub")
                  for l in range(L):
                      nc.tensor.matmul(ps_ub[:], W[l][:], child[:, :, l, p0:p1],
                                       start=(l == 0), stop=(l == 3))
                  off = OFF_LVL[j]
                  tmp = wrk.tile([128, Pc], f32, tag="tmp")
                  nc.vector.tensor_tensor(tmp[:], emis[:, off + p0:off + p1],
                                          ps_ub[:, 1, :], ALU.mult)
                  nc.vector.reciprocal(bnr[j][:, p0:p1], ps_ub[:, 1, :])
                  nc.scalar.copy(out=pb[j][:, 0, p0:p1], in_=ps_ub[:, 0, :])
                  ps_n = ps.tile([128, Pc], f32, tag="cs")
                  nc.tensor.matmul(ps_n[:], mbd[:], tmp[:], start=True, stop=True)
                  rn = wrk.tile([128, Pc], f32, tag="rn")
                  nc.vector.reciprocal(rn[:], ps_n[:])
                  nc.vector.tensor_tensor(pb[j][:, 1, p0:p1], tmp[:], rn[:], ALU.mult)

              # ---------- AllGather the 16 subtree roots ----------
              if max_phase < 4:
                  nc.sync.dma_start(out=o_part, in_=out_part[:])
                  return
              ag_in = dram.tile([128, 4], f32)
              ag_out = dram.tile([8, 128, 4], f32)
              nc.sync.dma_start(out=ag_in[:], in_=pb[0][:])
              nc.gpsimd.collective_compute(
                  "AllGather", mybir.AluOpType.bypass,
                  replica_groups=[list(range(NCORES))],
                  ins=[ag_in[:].opt()], outs=[ag_out[:].opt()])
              agt = per.tile([128, 8, 4], f32, tag="agt")
              nc.sync.dma_start(out=agt[:], in_=ag_out[:].transpose([1, 0, 2]))

              # roots view: [:, q, l%2-block...]: V_l = [128, 2(q), 4(p)]
              def ag_view(l, q=None):
                  v = agt[:].rearrange("p (k m) (q r) -> p q r k m", m=2, q=2, r=2)
                  v = v[:, :, l % 2, :, l // 2]        # [128, 2, 4]
                  return v if q is None else v[:, q, :]

              # ---------- top upward: global levels 1 then 0 ----------
              if max_phase < 5:
                  nc.sync.dma_start(out=o_part, in_=out_part[:])
                  return
              ps_u1 = ps.tile([128, 2, 4], f32, tag="ub")
              for l in range(L):
                  nc.tensor.matmul(ps_u1[:], W[l][:], ag_view(l),
                                   start=(l == 0), stop=(l == 3))
              pb1 = per.tile([128, 2, 4], f32, tag="pb_t1")
              bnr1 = per.tile([128, 4], f32, tag="bnr_t1")
              tmp1 = wrk.tile([128, 4], f32, tag="tmp")
              nc.vector.tensor_tensor(tmp1[:], emis_t[:, 1:5], ps_u1[:, 1, :], ALU.mult)
              nc.vector.reciprocal(bnr1[:], ps_u1[:, 1, :])
              nc.scalar.copy(out=pb1[:, 0, :], in_=ps_u1[:, 0, :])
              ps_n1 = ps.tile([128, 4], f32, tag="cs")
              nc.tensor.matmul(ps_n1[:], mbd[:], tmp1[:], start=True, stop=True)
              rn1 = wrk.tile([128, 4], f32, tag="rn")
              nc.vector.reciprocal(rn1[:], ps_n1[:])
              nc.vector.tensor_tensor(pb1[:, 1, :], tmp1[:], rn1[:], ALU.mult)

              ps_u0 = ps.tile([128, 2, 1], f32, tag="ub")
              for l in range(L):
                  nc.tensor.matmul(ps_u0[:], W[l][:], pb1[:, :, l:l + 1],
                                   start=(l == 0), stop=(l == 3))
              beta0 = per.tile([128, 1], f32, tag="beta0")
              bnr0 = per.tile([128, 1], f32, tag="bnr_t0")
              tmp0 = wrk.tile([128, 1], f32, tag="tmp")
              nc.vector.tensor_tensor(tmp0[:], emis_t[:, 0:1], ps_u0[:, 1, :], ALU.mult)
              nc.vector.reciprocal(bnr0[:], ps_u0[:, 1, :])
              ps_n0 = ps.tile([128, 1], f32, tag="cs")
              nc.tensor.matmul(ps_n0[:], mbd[:], tmp0[:], start=True, stop=True)
              rn0 = wrk.tile([128, 1], f32, tag="rn")
              nc.vector.reciprocal(rn0[:], ps_n0[:])
              nc.vector.tensor_tensor(beta0[:], tmp0[:], rn0[:], ALU.mult)

              scr = per.tile([128, 2048], f32, tag="scr")

              def ttr(in0, in1, col, shape):
                  """sum(in0*in1) over free dims into out_part[:, col].
                  (InstTensorTensorReduce wedges the device on this runtime, so
                  use a mult into scratch + a separate free-dim reduce.)"""
                  n = int(np.prod(shape))
                  out_v = scr[:, :n]
                  if len(shape) == 2:
                      out_v = out_v.rearrange("p (a b) -> p a b", a=shape[0])
                  nc.vector.tensor_tensor(out_v, in0, in1, ALU.mult)
                  nc.vector.tensor_reduce(out_part[:, col:col + 1], scr[:, :n],
                                          axis=AX.X, op=ALU.add)

              # ---------- top downward ----------
              if max_phase < 6:
                  nc.sync.dma_start(out=o_part, in_=out_part[:])
                  return
              # eps(root)=beta0. b contribution of node 0:

              if max_phase == 6 and sub < 1:
                  nc.sync.dma_start(out=o_part, in_=out_part[:])
                  return
              ttr(beta0[:], logb_t[:, 0:1], COL_B_NODE0, [1])

              if max_phase == 6 and sub < 2:
                  nc.sync.dma_start(out=o_part, in_=out_part[:])
                  return
              pe0 = wrk.tile([128, 1], f32, tag="pe_t")
              nc.vector.tensor_tensor(pe0[:], beta0[:], bnr0[:], ALU.mult)

              if max_phase == 6 and sub < 3:
                  nc.sync.dma_start(out=o_part, in_=out_part[:])
                  return
              ps_m0 = ps.tile([128, 4, 1], f32, tag="m")
              ps_q0 = ps.tile([128, 4, 1], f32, tag="mp")
              for l in range(L):
                  nc.tensor.matmul(ps_m0[:, l, :], W[l][:], pb1[:, 1, l:l + 1],
                                   start=True, stop=True)
                  nc.tensor.matmul(ps_q0[:, l, :], V[l][:], pb1[:, 1, l:l + 1],
                                   start=True, stop=True)

              if max_phase == 6 and sub < 4:
                  nc.sync.dma_start(out=o_part, in_=out_part[:])
                  return
              eps1 = per.tile([128, 4], f32, tag="eps_t1")
              nc.vector.tensor_tensor(
                  eps1[:].rearrange("p (n l) -> p n l", l=4),
                  ps_m0[:].transpose([0, 2, 1]),
                  pe0[:][:, :, None].to_broadcast([128, 1, 4]), ALU.mult)

              if max_phase == 6 and sub < 5:
                  nc.sync.dma_start(out=o_part, in_=out_part[:])
                  return
              ttr(ps_q0[:].rearrange("p a b -> p (a b)"),
                  pe0[:].to_broadcast([128, 4]), COL_A_TOP0, [4])
              nc.vector.tensor_reduce(
                  out_part[:, COL_RHO_TOP0:COL_RHO_TOP0 + 4],
                  eps1[:].rearrange("p (n l) -> p l n", l=4), axis=AX.X, op=ALU.add)
              ttr(eps1[:], logb_t[:, 1:5], COL_B_N14, [4])


              if max_phase == 6 and sub < 6:
                  nc.sync.dma_start(out=o_part, in_=out_part[:])
                  return
              # level 1 -> eps of the 16 roots
              pe1 = wrk.tile([128, 4], f32, tag="pe_t")
              nc.vector.tensor_tensor(pe1[:], eps1[:], bnr1[:], ALU.mult)
              ps_m1 = ps.tile([128, 4, 4], f32, tag="m")
              ps_q1 = ps.tile([128, 4, 4], f32, tag="mp")
              for l in range(L):
                  nc.tensor.matmul(ps_m1[:, l, :], W[l][:], ag_view(l, 1),
                                   start=True, stop=True)
                  nc.tensor.matmul(ps_q1[:, l, :], V[l][:], ag_view(l, 1),
                                   start=True, stop=True)

              if max_phase == 6 and sub < 7:
                  nc.sync.dma_start(out=o_part, in_=out_part[:])
                  return
              eps2 = per.tile([128, 16], f32, tag="eps_t2")
              nc.vector.tensor_tensor(
                  eps2[:].rearrange("p (n l) -> p n l", l=4),
                  ps_m1[:].transpose([0, 2, 1]),
                  pe1[:][:, :, None].to_broadcast([128, 4, 4]), ALU.mult)
              ttr(ps_q1[:], pe1[:][:, None, :].to_broadcast([128, 4, 4]),
                  COL_A_TOP1, [4, 4])
              nc.vector.tensor_reduce(
                  out_part[:, COL_RHO_TOP1:COL_RHO_TOP1 + 4],
                  eps2[:].rearrange("p (n l) -> p l n", l=4), axis=AX.X, op=ALU.add)
              ttr(eps2[:], logb_t[:, 5:21], COL_B_ROOTS, [16])


              if max_phase == 6 and sub < 8:
                  nc.sync.dma_start(out=o_part, in_=out_part[:])
                  return
              # pick this core's 2 roots
              e2l = per.tile([128, 4], f32, tag="e2l")
              nc.gpsimd.indirect_copy(e2l[:], eps2[:], ixl[:], True)
              nc.vector.tensor_copy(out=eps[0][:], in_=e2l[:, 0:2])

              # ---------- subtree downward: local levels j=0..4 ----------
              if max_phase < 7:
                  nc.sync.dma_start(out=o_part, in_=out_part[:])
                  return
              pe_t = [per.tile([128, P_LVL[j]], f32, tag=f"pe{j}", name=f"pe{j}") for j in range(5)]
              for u, (j, p0, p1) in enumerate(DN_UNITS):
                  Pc = p1 - p0
                  if p0 == 0:
                      nc.vector.tensor_tensor(pe_t[j][:], eps[j][:], bnr[j][:], ALU.mult)
                  ps_m = ps.tile([128, 4, Pc], f32, tag="m")
                  ps_q = ps.tile([128, 4, Pc], f32, tag="mp")
                  child_b = pb[j + 1][:, 1, :].rearrange("p (n l) -> p l n", l=4)
                  for l in range(L):
                      nc.tensor.matmul(ps_m[:, l, :], W[l][:], child_b[:, l, p0:p1],
                                       start=True, stop=True)
                      nc.tensor.matmul(ps_q[:, l, :], V[l][:], child_b[:, l, p0:p1],
                                       start=True, stop=True)
                  pe_b = pe_t[j][:, p0:p1][:, :, None].to_broadcast([128, Pc, 4])
                  nc.vector.tensor_tensor(
                      eps[j + 1][:, 4 * p0:4 * p1].rearrange("p (n l) -> p n l", l=4),
                      ps_m[:].transpose([0, 2, 1]), pe_b, ALU.mult)
                  ttr(ps_q[:], pe_t[j][:, p0:p1][:, None, :].to_broadcast([128, 4, Pc]),
                      COL_A_SUB + u, [4, Pc])
                  nc.vector.tensor_reduce(
                      out_part[:, COL_RHO_SUB + 4 * u:COL_RHO_SUB + 4 * u + 4],
                      eps[j + 1][:, 4 * p0:4 * p1].rearrange("p (n l) -> p l n", l=4),
                      axis=AX.X, op=ALU.add)
                  off = OFF_LVL[j + 1]
                  ttr(eps[j + 1][:, 4 * p0:4 * p1],
                      logb_s[:, off + 4 * p0:off + 4 * p1], COL_B_SUB + u, [4 * Pc])

              # pi term over leaves
              lp_per = log_pi[:][:, None, :].to_broadcast([128, 512, 4])
              ttr(eps[5][:].rearrange("p (r l) -> p r l", l=4), lp_per,
                  COL_PI, [512, 4])

              nc.sync.dma_start(out=o_part, in_=out_part[:])

              if debug:
                  for j in range(6):
                      d = dbg(f"pb{j}", [128, 2 * P_LVL[j]])
                      nc.sync.dma_start(out=d, in_=pb[j][:].rearrange("p a b -> p (a b)"))
                      d = dbg(f"eps{j}", [128, P_LVL[j]])
                      nc.sync.dma_start(out=d, in_=eps[j][:])
                  for nm, t_ in [("sm_a", sm_a), ("sm_b", sm_b), ("sm_pi", sm_pi),
                                 ("sm_sp", sm_sp), ("beta0", beta0), ("eps_t1", eps1),
                                 ("eps_t2", eps2), ("emis", emis)]:
                      d = dbg(nm, list(t_[:].shape))
                      nc.sync.dma_start(out=d, in_=t_[:])

            _emit()

    nc.finalize()
    return nc, dbg_out


def _shard_inputs(t, a, b, pi, sp):
    """Host-side prep of all per-core device inputs."""
    a = np.asarray(a, dtype=np.float32)
    b = np.asarray(b, dtype=np.float32)
    pi = np.asarray(pi, dtype=np.float32)
    sp = np.asarray(sp, dtype=np.float32)
    cores, idx_top = _host_prep(t)

    a_perm = np.ascontiguousarray(a.transpose(0, 2, 1, 3)).reshape(128, 32)
    b_gc = b.reshape(128, 256)
    pi_gc = pi.transpose(0, 1, 2).reshape(128, 4)
    sp_bc = np.repeat(sp, 8, axis=0).astype(np.float32)          # [(g,j), l]
    mbd = np.kron(np.eye(G, dtype=np.float32), np.ones((C, C), np.float32))

    in_maps = []
    for k in range(NCORES):
        in_maps.append({
            "a_perm": a_perm, "b_gc": b_gc, "pi_gc": pi_gc, "sp_bc": sp_bc,
            "mbd": mbd, "idx_sub": cores[k]["idx_sub"], "idx_top": idx_top,
            "idx_sel": cores[k]["idx_sel"],
        })
    return in_maps


def _combine(results, sp):
    """Host reduction of per-core partial columns into the [G] output."""
    sp = np.asarray(sp, dtype=np.float32)
    e = np.exp(sp - sp.max(axis=1, keepdims=True))
    log_sp = np.log(e / e.sum(axis=1, keepdims=True))            # [16, 4]

    S = [r["out_part"].reshape(G, C, NCOLS).sum(axis=1) for r in results]  # [16,64]
    tot_sub = sum(S)          # summed over cores
    S0 = S[0]

    a_lh = tot_sub[:, COL_A_SUB:COL_A_SUB + 8].sum(1) + S0[:, COL_A_TOP0] + S0[:, COL_A_TOP1]
    rho = tot_sub[:, COL_RHO_SUB:COL_RHO_SUB + 32].reshape(G, 8, L).sum(1) \
        + S0[:, COL_RHO_TOP0:COL_RHO_TOP0 + 4] + S0[:, COL_RHO_TOP1:COL_RHO_TOP1 + 4]
    sp_lh = (rho * log_sp).sum(1)
    b_lh = tot_sub[:, COL_B_SUB:COL_B_SUB + 8].sum(1) \
        + S0[:, COL_B_NODE0] + S0[:, COL_B_N14] + S0[:, COL_B_ROOTS]
    pi_lh = tot_sub[:, COL_PI]
    return (a_lh + b_lh + pi_lh + sp_lh).astype(np.float32)


_NC_CACHE = {}


def kernel(t, t_limits, a, b, pi, sp):
    from concourse.bass_utils import run_bass_kernel_spmd
    if "nc" not in _NC_CACHE:
        _NC_CACHE["nc"], _ = build_bass(debug=False)
    nc = _NC_CACHE["nc"]
    in_maps = _shard_inputs(t, a, b, pi, sp)
    res = run_bass_kernel_spmd(nc, in_maps, list(range(NCORES)))
    return _combine(res.results, sp)



# revision 4
# speedup vs baseline: 4.4059x; 4.4059x over previous
"""Bass/Trainium2 kernel for nn_BottomUpHTMM (bottom-up hidden tree Markov model).

Tree: complete 4-ary, depth 7, 21845 nodes. G=16 models, C=8 states, 256 labels.

v2 design:
- Each core owns 32 independent depth-3 subtrees rooted at the 256 level-4
  nodes (85..340): 32*(1+4+16+64) = 2720 nodes/core. Partition dim = (g,c).
- The prior chain of the reference cancels algebraically
  (prior*beta_il == A@beta_children), so only beta is propagated.
- Downward pass is factorized: eps(n) = eps(root_b) * D(n) where
  D(child) = D(parent)*bnr(parent)*m(child), bnr = 1/(A@beta_ch),
  m(child) = (W_l @ beta)(child). D is independent of anything above the
  subtree root, so each core emits per-root partial sums
  S_b = sum D*logb, S_A = sum Db*q, S_rho[l] = sum_{slot l} D, plus the root
  betas. No collective: the host computes the 341-node tree top and contracts
  eps(root) with the S terms.
- Emissions come from one ap_gather per buffer out of a host-built 1280-entry
  table (cols 0..255 = sm_b; cols 256+256*pos+label = leaf beta pre-normalized
  with sm_pi folded in). logb likewise (leaf entries include log pi).
- Reciprocals are computed as Exp(-Ln(x)) on the Scalar engine (DVE reciprocal
  is ~8 cycles/elem).

Per-core column layout: 32 blocks * 88 cols; block = [3 pad | root | 4 L1 |
16 L2 | 64 leaves]. Children of block-col c level j at co[j+1]+4*p+l,
co = [3, 4, 8, 24].
"""

import numpy as np

G = 16
C = 8
M = 256
L = 4
NCORES = 8
T_SIZE = 21845
NBLK = 32                 # subtrees per core
BLK = 88                  # cols per block (3 pad + 85 nodes)
NCOL = NBLK * BLK         # 2816
CO = [3, 4, 8, 24]        # level col offsets within block
O21 = [0, 1, 5]           # level offsets within the 21 internal slots
NB = [1, 4, 16]           # parents per block per level
NIDX = NCOL // 16         # 176

# out cols
OC_RB = 0     # 0..32   root beta
OC_SB = 32    # 32..64  S_b
OC_SA = 64    # 64..96  S_A
OC_RHO = 96   # 96..224 S_rho (block-major x l)
NOUT = 224


def _softmax(x, axis):
    e = np.exp(x - x.max(axis=axis, keepdims=True))
    return e / e.sum(axis=axis, keepdims=True)


def _wrap_idx(idx):
    """idx j at partition j%16, slot j//16, replicated across 8 gpsimd cores."""
    idx = np.asarray(idx, dtype=np.int16)
    n = len(idx)
    assert n % 16 == 0
    grid = idx.reshape(n // 16, 16).T          # [16, n/16]
    return np.tile(grid, (8, 1))               # [128, n/16]


def _block_cols():
    """Per-block arrays: node id builder + (col -> level, pos-slot)."""
    # block-relative heap ids per col (0 for pads), using cols 3..88
    rel = np.full(BLK, -1, np.int64)
    rel[3] = 0
    for p in range(21):                        # internal block-rel heap ids 0..20
        for l in range(4):
            rel[CO[1] + 4 * p + l if p == 0 else 0] = 0
    # simpler: levels explicitly
    rel[3] = 0
    rel[4:8] = np.arange(1, 5)
    rel[8:24] = np.arange(5, 21)
    rel[24:88] = np.arange(21, 85)
    return rel


_REL = _block_cols()


def _host_prep(t, a, b, pi, sp):
    t = np.asarray(t)
    labels = t[:, 0].astype(np.int64)
    a = np.asarray(a, np.float64)
    b = np.asarray(b, np.float64)
    pi = np.asarray(pi, np.float64)
    sp = np.asarray(sp, np.float64)
    sm_a = _softmax(a, 1)
    sm_b = _softmax(b, 2)
    sm_pi = _softmax(pi, 1)
    sm_sp = _softmax(sp, 1)
    asp = sm_a * sm_sp[:, None, None, :]

    # tables [128, 1280]
    tb = np.zeros((128, 1280), np.float32)
    tbl = np.zeros((128, 1280), np.float32)
    sb128 = sm_b.reshape(128, M)
    tb[:, :M] = sb128
    tbl[:, :M] = np.log(sb128)
    for pos in range(4):
        v = sm_b * sm_pi[:, :, pos][:, :, None]          # [G,C,M]
        s = v.sum(1, keepdims=True)
        tb[:, M + M * pos:M + M * (pos + 1)] = (v / s).reshape(128, M)
        tbl[:, M + M * pos:M + M * (pos + 1)] = np.log(v).reshape(128, M)

    # weights [128, 1152]: W0..3 V0..3 mbd
    la = np.log(sm_a)
    wv = np.zeros((128, 1152), np.float32)
    for l in range(4):
        Wl = np.zeros((128, 128))
        Vl = np.zeros((128, 128))
        for g in range(G):
            Wl[g * C:(g + 1) * C, g * C:(g + 1) * C] = asp[g, :, :, l].T      # [j,i]
            Vl[g * C:(g + 1) * C, g * C:(g + 1) * C] = (asp * la)[g, :, :, l].T
        wv[:, 128 * l:128 * (l + 1)] = Wl
        wv[:, 512 + 128 * l:512 + 128 * (l + 1)] = Vl
    wv[:, 1024:1152] = np.kron(np.eye(G), np.ones((C, C)))

    # per-core node ids + gather idx
    # block-rel heap: node 0 root, children of p at 4p+1+l
    gid_rel = np.zeros(85, np.int64)
    cores = []
    gids = []
    for k in range(NCORES):
        idx = np.zeros(NCOL, np.int64)
        gid_all = np.zeros((NBLK, 85), np.int64)
        for bq in range(NBLK):
            root = 85 + NBLK * k + bq
            gid_rel[0] = root
            for p in range(21):
                for l in range(4):
                    gid_rel[4 * p + 1 + l] = 4 * gid_rel[p] + 1 + l
            gid_all[bq] = gid_rel
            base = BLK * bq
            lab = labels[gid_rel]
            idx[base + 3:base + 24] = lab[:21]                     # internal: sm_b
            pos = (gid_rel[21:] - 1) % 4
            idx[base + 24:base + 88] = M + M * pos + lab[21:]      # leaves
        ebd = tb[:, idx].astype(np.float32)
        lgbd = tbl[:, idx].astype(np.float32)
        for bq in range(NBLK):
            ebd[:, BLK * bq:BLK * bq + 3] = 1.0
            lgbd[:, BLK * bq:BLK * bq + 3] = 0.0
        cores.append((ebd, lgbd))
        gids.append(gid_all)

    host = dict(labels=labels, asp=asp, sm_b=sm_b, sm_pi=sm_pi,
                log_a=la, log_b=np.log(sm_b), log_sp=np.log(sm_sp))
    return tb, tbl, wv, cores, gids, host


def _combine(results, host):
    """Host: 341-node tree top + contraction with per-core S terms."""
    labels = host["labels"]; asp = host["asp"]; sm_b = host["sm_b"]
    log_a = host["log_a"]; log_b = host["log_b"]; log_sp = host["log_sp"]

    beta = np.zeros((341, G, C))
    Ab = np.zeros((85, G, C))
    for k in range(NCORES):
        rb = np.asarray(results[k]["out"], np.float64)
        beta[85 + NBLK * k:85 + NBLK * (k + 1)] = \
            rb[:, OC_RB:OC_RB + NBLK].T.reshape(NBLK, G, C)
    for lev in range(3, -1, -1):
        s, e = (4 ** lev - 1) // 3, (4 ** (lev + 1) - 1) // 3
        ch = 4 * np.arange(s, e)[:, None] + np.arange(1, 5)[None, :]
        AbP = np.einsum('gijl,plgj->pgi', asp, beta[ch])
        tmp = np.einsum('gcp,pgc->pgc', sm_b[:, :, labels[s:e]], AbP)
        beta[s:e] = tmp / tmp.sum(2, keepdims=True)
        Ab[s:e] = AbP
    eps = np.zeros((341, G, C)); eps[0] = beta[0]
    a_lh = np.zeros(G); rho = np.zeros((G, L))
    for lev in range(0, 4):
        s, e = (4 ** lev - 1) // 3, (4 ** (lev + 1) - 1) // 3
        ch = 4 * np.arange(s, e)[:, None] + np.arange(1, 5)[None, :]
        pe = eps[s:e] / Ab[s:e]
        mch = np.einsum('gijl,plgj->pgil', asp, beta[ch])
        epsc = pe[:, :, :, None] * mch
        for l in range(4):
            eps[ch[:, l]] = epsc[:, :, :, l]
        rho += epsc.sum(2).sum(0)
        a_lh += np.einsum('pgi,gijl,gijl,plgj->g', pe, asp, log_a, beta[ch])
    b_lh = np.einsum('ugc,gcu->g', eps, log_b[:, :, labels[:341]])

    # device terms
    for k in range(NCORES):
        out = np.asarray(results[k]["out"], np.float64)
        er = eps[85 + NBLK * k:85 + NBLK * (k + 1)].reshape(NBLK, 128)  # [b,(g,c)]
        S_b = out[:, OC_SB:OC_SB + NBLK].T          # [b, 128]
        S_A = out[:, OC_SA:OC_SA + NBLK].T
        S_r = out[:, OC_RHO:OC_RHO + 4 * NBLK].T.reshape(NBLK, 4, 128)
        b_lh += (er * S_b).reshape(NBLK, G, C).sum(0).sum(1)
        a_lh += (er * S_A).reshape(NBLK, G, C).sum(0).sum(1)
        rho += np.einsum('blp,bp->pl', S_r, er).reshape(G, C, L).sum(1)
    sp_lh = (rho * log_sp).sum(1)
    return (a_lh + b_lh + sp_lh).astype(np.float32)


def build_bass():
    import concourse.bacc as bacc
    import concourse.tile as tile
    import concourse.mybir as mybir
    from concourse import bass

    f32 = mybir.dt.float32
    i16 = mybir.dt.int16
    AF = mybir.ActivationFunctionType
    ALU = mybir.AluOpType
    AX = mybir.AxisListType

    nc = bacc.Bacc("TRN2", target_bir_lowering=False, debug=False,
                   num_devices=NCORES)

    eb_in = nc.dram_tensor("ebd", [128, NCOL], f32, kind="ExternalInput").ap()
    lgb_in = nc.dram_tensor("lgbd", [128, NCOL], f32, kind="ExternalInput").ap()
    wv_in = nc.dram_tensor("wv", [128, 1152], f32, kind="ExternalInput").ap()
    o_out = nc.dram_tensor("out", [128, NOUT], f32, kind="ExternalOutput").ap()

    with tile.TileContext(nc) as tc:
        with tc.tile_pool(name="per", bufs=1) as per, \
             tc.tile_pool(name="wrk", bufs=2) as wrk, \
             tc.tile_pool(name="ps", bufs=2, space="PSUM") as ps:
            psm = ps

            wv = per.tile([128, 1152], f32, tag="wv")

            W = [wv[:, 128 * l:128 * (l + 1)] for l in range(4)]
            V = [wv[:, 512 + 128 * l:512 + 128 * (l + 1)] for l in range(4)]
            mbd = wv[:, 1024:1152]

            eb = per.tile([128, NCOL], f32, tag="eb")       # emission -> beta
            lgb = per.tile([128, NCOL], f32, tag="lgb")     # log emission
            nc.sync.dma_start(out=eb[:, :NCOL // 2], in_=eb_in[:, :NCOL // 2])
            nc.sync.dma_start(out=eb[:, NCOL // 2:], in_=eb_in[:, NCOL // 2:])
            nc.sync.dma_start(out=wv[:], in_=wv_in)
            nc.sync.dma_start(out=lgb[:], in_=lgb_in)
            Dt = per.tile([128, NCOL], f32, tag="Dt")       # eps factor D
            sbp = per.tile([128, NCOL], f32, tag="sbp")     # D*logb scratch
            bnr = per.tile([128, NBLK, 21], f32, tag="bnr")
            SA = per.tile([128, NBLK, 21], f32, tag="SA")
            outp = per.tile([128, NOUT], f32, tag="outp")

            ebv = eb[:].rearrange("p (b c) -> p b c", b=NBLK)
            Dv = Dt[:].rearrange("p (b c) -> p b c", b=NBLK)

            def child_view(b0, nbl, j, l):
                """beta of l-th children of level-j parents: [128, nbl, NB[j]]"""
                v = ebv[:, b0:b0 + nbl, CO[j + 1]:CO[j + 1] + 4 * NB[j]]
                return v.rearrange("p b (n l) -> p b n l", l=4)[:, :, :, l]

            # ---------------- upward ----------------
            for j, chunks in [(2, [(0, 16), (16, 16)]), (1, [(0, 32)]), (0, [(0, 32)])]:
                n_b = NB[j]
                for (b0, nbl) in chunks:
                    N = nbl * n_b
                    ub = ps.tile([128, nbl, n_b], f32, tag="ub")
                    for l in range(4):
                        nc.tensor.matmul(ub[:], W[l], child_view(b0, nbl, j, l),
                                         start=(l == 0), stop=(l == 3))
                    # bnr = exp(-ln(A@beta))
                    lnb = wrk.tile([128, nbl, n_b], f32, tag="lnb")
                    nc.scalar.activation(out=lnb[:], in_=ub[:], func=AF.Ln)
                    nc.scalar.activation(
                        out=bnr[:, b0:b0 + nbl, O21[j]:O21[j] + n_b],
                        in_=lnb[:], func=AF.Exp, scale=-1.0)
                    # tmp = emis * (A@beta);  beta = tmp * exp(-ln(colsum))
                    tmp = wrk.tile([128, nbl, n_b], f32, tag="tmp")
                    nc.vector.tensor_tensor(
                        tmp[:], ebv[:, b0:b0 + nbl, CO[j]:CO[j] + n_b], ub[:],
                        ALU.mult)
                    pn = ps.tile([128, nbl, n_b], f32, tag="pn")
                    nc.tensor.matmul(pn[:], mbd,
                                     tmp[:].rearrange("p b n -> p (b n)"),
                                     start=True, stop=True)
                    lnn = wrk.tile([128, nbl, n_b], f32, tag="lnn")
                    nc.scalar.activation(out=lnn[:], in_=pn[:], func=AF.Ln)
                    rn = wrk.tile([128, nbl, n_b], f32, tag="rn")
                    nc.scalar.activation(out=rn[:], in_=lnn[:], func=AF.Exp,
                                         scale=-1.0)
                    nc.vector.tensor_tensor(
                        ebv[:, b0:b0 + nbl, CO[j]:CO[j] + n_b], tmp[:], rn[:],
                        ALU.mult)

            # root betas to output
            nc.scalar.copy(out=outp[:, OC_RB:OC_RB + NBLK], in_=ebv[:, :, 3])

            # ---------------- downward (D chain + S terms) ----------------
            Db0 = bnr[:, :, 0]                                   # [128, 32]
            # j = 0
            m0 = psm.tile([128, 4, NBLK], f32, tag="m")
            for l in range(4):
                nc.tensor.matmul(m0[:, l, :], W[l],
                                 child_view(0, NBLK, 0, l)
                                 .rearrange("p b n -> p (b n)"),
                                 start=True, stop=True)
            q0 = psm.tile([128, NBLK], f32, tag="q")
            for l in range(4):
                nc.tensor.matmul(q0[:], V[l],
                                 child_view(0, NBLK, 0, l)
                                 .rearrange("p b n -> p (b n)"),
                                 start=(l == 0), stop=(l == 3))
            nc.vector.tensor_tensor(Dv[:, :, 4:8], m0[:].transpose([0, 2, 1]),
                                    Db0[:, :, None].to_broadcast([128, NBLK, 4]),
                                    ALU.mult)
            nc.vector.tensor_tensor(SA[:, :, 0:1], Db0[:, :, None], q0[:, :, None],
                                    ALU.mult)
            # j = 1
            Db1 = wrk.tile([128, NBLK, 4], f32, tag="db1")
            nc.vector.tensor_tensor(Db1[:], Dv[:, :, 4:8], bnr[:, :, 1:5], ALU.mult)
            m1 = psm.tile([128, 4, NBLK, 4], f32, tag="m")
            for l in range(4):
                nc.tensor.matmul(m1[:, l, :, :], W[l], child_view(0, NBLK, 1, l),
                                 start=True, stop=True)
            q1 = psm.tile([128, NBLK, 4], f32, tag="q")
            for l in range(4):
                nc.tensor.matmul(q1[:], V[l], child_view(0, NBLK, 1, l),
                                 start=(l == 0), stop=(l == 3))
            nc.vector.tensor_tensor(
                Dv[:, :, 8:24].rearrange("p b (n l) -> p b n l", l=4),
                m1[:].transpose([0, 2, 3, 1]),
                Db1[:, :, :, None].to_broadcast([128, NBLK, 4, 4]), ALU.mult)
            nc.vector.tensor_tensor(SA[:, :, 1:5], Db1[:], q1[:], ALU.mult)
            # j = 2
            Db2 = wrk.tile([128, NBLK, 16], f32, tag="db2")
            nc.vector.tensor_tensor(Db2[:], Dv[:, :, 8:24], bnr[:, :, 5:21],
                                    ALU.mult)
            q2 = psm.tile([128, NBLK, 16], f32, tag="q")
            for l in range(4):
                nc.tensor.matmul(q2[:], V[l], child_view(0, NBLK, 2, l),
                                 start=(l == 0), stop=(l == 3))
            nc.vector.tensor_tensor(SA[:, :, 5:21], Db2[:], q2[:], ALU.mult)
            for b0 in range(0, NBLK, 8):
                m2 = psm.tile([128, 4, 8, 16], f32, tag="m")
                for l in range(4):
                    nc.tensor.matmul(m2[:, l, :, :], W[l], child_view(b0, 8, 2, l),
                                     start=True, stop=True)
                nc.vector.tensor_tensor(
                    Dv[:, b0:b0 + 8, 24:88].rearrange("p b (n l) -> p b n l", l=4),
                    m2[:].transpose([0, 2, 3, 1]),
                    Db2[:, b0:b0 + 8, :, None].to_broadcast([128, 8, 16, 4]),
                    ALU.mult)

            # ---------------- global reductions ----------------
            nc.vector.tensor_tensor(sbp[:], Dt[:], lgb[:], ALU.mult)
            sbv = sbp[:].rearrange("p (b c) -> p b c", b=NBLK)
            nc.vector.tensor_reduce(outp[:, OC_SB:OC_SB + NBLK],
                                    sbv[:, :, 4:88], axis=AX.X, op=ALU.add)
            nc.vector.tensor_reduce(outp[:, OC_SA:OC_SA + NBLK],
                                    SA[:], axis=AX.X, op=ALU.add)
            rhov = Dv[:, :, 4:88].rearrange("p b (n l) -> p b l n", l=4)
            nc.vector.tensor_reduce(
                outp[:, OC_RHO:OC_RHO + 4 * NBLK]
                .rearrange("p (b l) -> p b l", l=4),
                rhov, axis=AX.X, op=ALU.add)

            nc.sync.dma_start(out=o_out, in_=outp[:])

    nc.finalize()
    return nc


_NC_CACHE = {}


def _shard_inputs(t, a, b, pi, sp):
    tb, tbl, wv, cores, gids, host = _host_prep(t, a, b, pi, sp)
    in_maps = []
    for k in range(NCORES):
        in_maps.append({"ebd": cores[k][0], "lgbd": cores[k][1], "wv": wv})
    return in_maps, host


def kernel(t, t_limits, a, b, pi, sp):
    from concourse.bass_utils import run_bass_kernel_spmd
    if "nc" not in _NC_CACHE:
        _NC_CACHE["nc"] = build_bass()
    nc = _NC_CACHE["nc"]
    in_maps, host = _shard_inputs(t, a, b, pi, sp)
    res = run_bass_kernel_spmd(nc, in_maps, list(range(NCORES)))
    return _combine(res.results, host)


# revision 6
# speedup vs baseline: 7.7388x; 1.7565x over previous
"""Bass/Trainium2 kernel for nn_BottomUpHTMM (bottom-up hidden tree Markov model).

Tree: complete 4-ary, depth 7, 21845 nodes. G=16 models, C=8 states, 256 labels.

v2 design:
- Each core owns 32 independent depth-3 subtrees rooted at the 256 level-4
  nodes (85..340): 32*(1+4+16+64) = 2720 nodes/core. Partition dim = (g,c).
- The prior chain of the reference cancels algebraically
  (prior*beta_il == A@beta_children), so only beta is propagated.
- Downward pass is factorized: eps(n) = eps(root_b) * D(n) where
  D(child) = D(parent)*bnr(parent)*m(child), bnr = 1/(A@beta_ch),
  m(child) = (W_l @ beta)(child). D is independent of anything above the
  subtree root, so each core emits per-root partial sums
  S_b = sum D*logb, S_A = sum Db*q, S_rho[l] = sum_{slot l} D, plus the root
  betas. No collective: the host computes the 341-node tree top and contracts
  eps(root) with the S terms.
- Emissions come from one ap_gather per buffer out of a host-built 1280-entry
  table (cols 0..255 = sm_b; cols 256+256*pos+label = leaf beta pre-normalized
  with sm_pi folded in). logb likewise (leaf entries include log pi).
- Reciprocals are computed as Exp(-Ln(x)) on the Scalar engine (DVE reciprocal
  is ~8 cycles/elem).

Per-core column layout: 32 blocks * 88 cols; block = [3 pad | root | 4 L1 |
16 L2 | 64 leaves]. Children of block-col c level j at co[j+1]+4*p+l,
co = [3, 4, 8, 24].
"""

import numpy as np
import ml_dtypes

BF16 = ml_dtypes.bfloat16

G = 16
C = 8
M = 256
L = 4
NCORES = 8
T_SIZE = 21845
NBLK = 32                 # subtrees per core
BLK = 88                  # cols per block (3 pad + 85 nodes)
NCOL = NBLK * BLK         # 2816
CO = [3, 4, 8, 24]        # level col offsets within block
O21 = [0, 1, 5]           # level offsets within the 21 internal slots
NB = [1, 4, 16]           # parents per block per level
NIDX = NCOL // 16         # 176

# out cols
OC_RB = 0     # 0..32   root beta
OC_SB = 32    # 32..64  S_b
OC_SA = 64    # 64..96  S_A
OC_RHO = 96   # 96..224 S_rho (block-major x l)
NOUT = 224


def _softmax(x, axis):
    e = np.exp(x - x.max(axis=axis, keepdims=True))
    return e / e.sum(axis=axis, keepdims=True)


def _wrap_idx(idx):
    """idx j at partition j%16, slot j//16, replicated across 8 gpsimd cores."""
    idx = np.asarray(idx, dtype=np.int16)
    n = len(idx)
    assert n % 16 == 0
    grid = idx.reshape(n // 16, 16).T          # [16, n/16]
    return np.tile(grid, (8, 1))               # [128, n/16]


def _block_cols():
    """Per-block arrays: node id builder + (col -> level, pos-slot)."""
    # block-relative heap ids per col (0 for pads), using cols 3..88
    rel = np.full(BLK, -1, np.int64)
    rel[3] = 0
    for p in range(21):                        # internal block-rel heap ids 0..20
        for l in range(4):
            rel[CO[1] + 4 * p + l if p == 0 else 0] = 0
    # simpler: levels explicitly
    rel[3] = 0
    rel[4:8] = np.arange(1, 5)
    rel[8:24] = np.arange(5, 21)
    rel[24:88] = np.arange(21, 85)
    return rel


_REL = _block_cols()


def _host_prep(t, a, b, pi, sp):
    t = np.asarray(t)
    labels = t[:, 0].astype(np.int64)
    a = np.asarray(a, np.float64)
    b = np.asarray(b, np.float64)
    pi = np.asarray(pi, np.float64)
    sp = np.asarray(sp, np.float64)
    sm_a = _softmax(a, 1)
    sm_b = _softmax(b, 2)
    sm_pi = _softmax(pi, 1)
    sm_sp = _softmax(sp, 1)
    asp = sm_a * sm_sp[:, None, None, :]

    # tables [128, 1280]
    tb = np.zeros((128, 1280), np.float32)
    tbl = np.zeros((128, 1280), np.float32)
    sb128 = sm_b.reshape(128, M)
    tb[:, :M] = sb128
    tbl[:, :M] = np.log(sb128)
    for pos in range(4):
        v = sm_b * sm_pi[:, :, pos][:, :, None]          # [G,C,M]
        s = v.sum(1, keepdims=True)
        tb[:, M + M * pos:M + M * (pos + 1)] = (v / s).reshape(128, M)
        tbl[:, M + M * pos:M + M * (pos + 1)] = np.log(v).reshape(128, M)

    # weights [128, 1152]: W0..3 V0..3 mbd
    la = np.log(sm_a)
    wv = np.zeros((128, 1024), np.float32)
    for l in range(4):
        Wl = np.zeros((128, 128))
        Vl = np.zeros((128, 128))
        for g in range(G):
            Wl[g * C:(g + 1) * C, g * C:(g + 1) * C] = asp[g, :, :, l].T      # [j,i]
            Vl[g * C:(g + 1) * C, g * C:(g + 1) * C] = (asp * la)[g, :, :, l].T
        wv[:, 128 * l:128 * (l + 1)] = Wl
        wv[:, 512 + 128 * l:512 + 128 * (l + 1)] = Vl

    wv = wv.astype(BF16)

    # per-core node ids + gather idx
    # block-rel heap: node 0 root, children of p at 4p+1+l
    gid_rel = np.zeros(85, np.int64)
    cores = []
    gids = []
    for k in range(NCORES):
        idx = np.zeros(NCOL, np.int64)
        gid_all = np.zeros((NBLK, 85), np.int64)
        for bq in range(NBLK):
            root = 85 + NBLK * k + bq
            gid_rel[0] = root
            for p in range(21):
                for l in range(4):
                    gid_rel[4 * p + 1 + l] = 4 * gid_rel[p] + 1 + l
            gid_all[bq] = gid_rel
            base = BLK * bq
            lab = labels[gid_rel]
            idx[base + 3:base + 24] = lab[:21]                     # internal: sm_b
            pos = (gid_rel[21:] - 1) % 4
            idx[base + 24:base + 88] = M + M * pos + lab[21:]      # leaves
        ebd = tb[:, idx].astype(np.float32)
        lgbd = tbl[:, idx].astype(np.float32)
        for bq in range(NBLK):
            ebd[:, BLK * bq:BLK * bq + 3] = 1.0
            lgbd[:, BLK * bq:BLK * bq + 3] = 0.0
        cores.append((ebd.astype(BF16), lgbd.astype(BF16)))
        gids.append(gid_all)

    host = dict(labels=labels, asp=asp, sm_b=sm_b, sm_pi=sm_pi,
                log_a=la, log_b=np.log(sm_b), log_sp=np.log(sm_sp))
    return tb, tbl, wv, cores, gids, host


def _combine(results, host):
    """Host: 341-node tree top + contraction with per-core S terms."""
    labels = host["labels"]; asp = host["asp"]; sm_b = host["sm_b"]
    log_a = host["log_a"]; log_b = host["log_b"]; log_sp = host["log_sp"]

    beta = np.zeros((341, G, C))
    Ab = np.zeros((85, G, C))
    for k in range(NCORES):
        rb = np.asarray(results[k]["out"], np.float64)
        rbm = rb[:, OC_RB:OC_RB + NBLK].T.reshape(NBLK, G, C)
        beta[85 + NBLK * k:85 + NBLK * (k + 1)] = \
            rbm / rbm.sum(2, keepdims=True)
    for lev in range(3, -1, -1):
        s, e = (4 ** lev - 1) // 3, (4 ** (lev + 1) - 1) // 3
        ch = 4 * np.arange(s, e)[:, None] + np.arange(1, 5)[None, :]
        AbP = np.einsum('gijl,plgj->pgi', asp, beta[ch])
        tmp = np.einsum('gcp,pgc->pgc', sm_b[:, :, labels[s:e]], AbP)
        beta[s:e] = tmp / tmp.sum(2, keepdims=True)
        Ab[s:e] = AbP
    eps = np.zeros((341, G, C)); eps[0] = beta[0]
    a_lh = np.zeros(G); rho = np.zeros((G, L))
    for lev in range(0, 4):
        s, e = (4 ** lev - 1) // 3, (4 ** (lev + 1) - 1) // 3
        ch = 4 * np.arange(s, e)[:, None] + np.arange(1, 5)[None, :]
        pe = eps[s:e] / Ab[s:e]
        mch = np.einsum('gijl,plgj->pgil', asp, beta[ch])
        epsc = pe[:, :, :, None] * mch
        for l in range(4):
            eps[ch[:, l]] = epsc[:, :, :, l]
        rho += epsc.sum(2).sum(0)
        a_lh += np.einsum('pgi,gijl,gijl,plgj->g', pe, asp, log_a, beta[ch])
    b_lh = np.einsum('ugc,gcu->g', eps, log_b[:, :, labels[:341]])

    # device terms
    for k in range(NCORES):
        out = np.asarray(results[k]["out"], np.float64)
        er = eps[85 + NBLK * k:85 + NBLK * (k + 1)].reshape(NBLK, 128)  # [b,(g,c)]
        S_b = out[:, OC_SB:OC_SB + NBLK].T          # [b, 128]
        S_A = out[:, OC_SA:OC_SA + NBLK].T
        S_r = out[:, OC_RHO:OC_RHO + 4 * NBLK].T.reshape(NBLK, 4, 128)
        b_lh += (er * S_b).reshape(NBLK, G, C).sum(0).sum(1)
        a_lh += (er * S_A).reshape(NBLK, G, C).sum(0).sum(1)
        rho += np.einsum('blp,bp->pl', S_r, er).reshape(G, C, L).sum(1)
    sp_lh = (rho * log_sp).sum(1)
    return (a_lh + b_lh + sp_lh).astype(np.float32)


def build_bass():
    import concourse.bacc as bacc
    import concourse.tile as tile
    import concourse.mybir as mybir
    from concourse import bass

    f32 = mybir.dt.float32
    bf16 = mybir.dt.bfloat16
    AF = mybir.ActivationFunctionType
    ALU = mybir.AluOpType
    AX = mybir.AxisListType

    nc = bacc.Bacc("TRN2", target_bir_lowering=False, debug=False,
                   num_devices=NCORES)

    eb_in = nc.dram_tensor("ebd", [128, NCOL], bf16, kind="ExternalInput").ap()
    lgb_in = nc.dram_tensor("lgbd", [128, NCOL], bf16, kind="ExternalInput").ap()
    wv_in = nc.dram_tensor("wv", [128, 1024], bf16, kind="ExternalInput").ap()
    o_out = nc.dram_tensor("out", [128, NOUT], f32, kind="ExternalOutput").ap()

    with tile.TileContext(nc) as tc:
        with tc.tile_pool(name="per", bufs=1) as per, \
             tc.tile_pool(name="wrk", bufs=2) as wrk, \
             tc.tile_pool(name="ps", bufs=2, space="PSUM") as ps:

            wv = per.tile([128, 1024], bf16, tag="wv")
            W = [wv[:, 128 * l:128 * (l + 1)] for l in range(4)]
            V = [wv[:, 512 + 128 * l:512 + 128 * (l + 1)] for l in range(4)]

            eb = per.tile([128, NCOL], bf16, tag="eb")      # emission -> beta
            lgb = per.tile([128, NCOL], bf16, tag="lgb")    # log emission
            nc.sync.dma_start(out=eb[:, :NCOL // 2], in_=eb_in[:, :NCOL // 2])
            nc.sync.dma_start(out=eb[:, NCOL // 2:], in_=eb_in[:, NCOL // 2:])
            nc.sync.dma_start(out=wv[:], in_=wv_in)
            nc.sync.dma_start(out=lgb[:], in_=lgb_in)
            Dt = per.tile([128, NCOL], bf16, tag="Dt")      # eps factor D
            sbp = per.tile([128, NCOL], bf16, tag="sbp")    # D*logb scratch
            bnr = per.tile([128, NBLK, 21], bf16, tag="bnr")
            SA = per.tile([128, NBLK, 21], f32, tag="SA")
            outp = per.tile([128, NOUT], f32, tag="outp")

            ebv = eb[:].rearrange("p (b c) -> p b c", b=NBLK)
            lgv = lgb[:].rearrange("p (b c) -> p b c", b=NBLK)
            Dv = Dt[:].rearrange("p (b c) -> p b c", b=NBLK)
            sbv = sbp[:].rearrange("p (b c) -> p b c", b=NBLK)

            def child_view(b0, nbl, j, l):
                """beta of l-th children of level-j parents: [128, nbl, NB[j]]"""
                v = ebv[:, b0:b0 + nbl, CO[j + 1]:CO[j + 1] + 4 * NB[j]]
                return v.rearrange("p b (n l) -> p b n l", l=4)[:, :, :, l]

            # ---------------- upward (no per-node normalization) ----------------
            lns = []
            for u, (j, b0, nbl) in enumerate([(2, 0, 16), (2, 16, 16),
                                              (1, 0, 32), (0, 0, 32)]):
                n_b = NB[j]
                ub = ps.tile([128, nbl, n_b], f32, tag="ub")
                for l in range(4):
                    nc.tensor.matmul(ub[:], W[l], child_view(b0, nbl, j, l),
                                     start=(l == 0), stop=(l == 3))
                ln = per.tile([128, nbl, n_b], f32, tag=f"ln{u}")
                nc.scalar.activation(out=ln[:], in_=ub[:], func=AF.Ln)
                nc.vector.tensor_tensor(
                    ebv[:, b0:b0 + nbl, CO[j]:CO[j] + n_b],
                    ebv[:, b0:b0 + nbl, CO[j]:CO[j] + n_b], ub[:], ALU.mult)
                lns.append((j, b0, nbl, ln))
            # bnr = exp(-ln(A@beta)); batched so ACT loads each table once
            for j, b0, nbl, ln in lns:
                nc.scalar.activation(out=bnr[:, b0:b0 + nbl, O21[j]:O21[j] + NB[j]],
                                     in_=ln[:], func=AF.Exp, scale=-1.0)

            # root (unnormalized) betas to output
            nc.scalar.copy(out=outp[:, OC_RB:OC_RB + NBLK], in_=ebv[:, :, 3])

            # ---------------- downward (D chain + S terms) ----------------
            Db0 = bnr[:, :, 0]                                   # [128, 32]
            # j = 0
            m0 = ps.tile([128, 4, NBLK], f32, tag="m")
            for l in range(4):
                nc.tensor.matmul(m0[:, l, :], W[l],
                                 child_view(0, NBLK, 0, l)
                                 .rearrange("p b n -> p (b n)"),
                                 start=True, stop=True)
            q0 = ps.tile([128, NBLK], f32, tag="q")
            for l in range(4):
                nc.tensor.matmul(q0[:], V[l],
                                 child_view(0, NBLK, 0, l)
                                 .rearrange("p b n -> p (b n)"),
                                 start=(l == 0), stop=(l == 3))
            nc.vector.tensor_tensor(Dv[:, :, 4:8], m0[:].transpose([0, 2, 1]),
                                    Db0[:, :, None].to_broadcast([128, NBLK, 4]),
                                    ALU.mult)
            nc.vector.tensor_tensor(SA[:, :, 0:1], Db0[:, :, None], q0[:, :, None],
                                    ALU.mult)
            # j = 1
            Db1 = wrk.tile([128, NBLK, 4], bf16, tag="db1")
            nc.vector.tensor_tensor(Db1[:], Dv[:, :, 4:8], bnr[:, :, 1:5], ALU.mult)
            m1 = ps.tile([128, 4, NBLK, 4], f32, tag="m")
            for l in range(4):
                nc.tensor.matmul(m1[:, l, :, :], W[l], child_view(0, NBLK, 1, l),
                                 start=True, stop=True)
            q1 = ps.tile([128, NBLK, 4], f32, tag="q")
            for l in range(4):
                nc.tensor.matmul(q1[:], V[l], child_view(0, NBLK, 1, l),
                                 start=(l == 0), stop=(l == 3))
            nc.vector.tensor_tensor(
                Dv[:, :, 8:24].rearrange("p b (n l) -> p b n l", l=4),
                m1[:].transpose([0, 2, 3, 1]),
                Db1[:, :, :, None].to_broadcast([128, NBLK, 4, 4]), ALU.mult)
            nc.vector.tensor_tensor(SA[:, :, 1:5], Db1[:], q1[:], ALU.mult)
            # j = 2
            Db2 = wrk.tile([128, NBLK, 16], bf16, tag="db2")
            nc.vector.tensor_tensor(Db2[:], Dv[:, :, 8:24], bnr[:, :, 5:21],
                                    ALU.mult)
            q2 = ps.tile([128, NBLK, 16], f32, tag="q")
            for l in range(4):
                nc.tensor.matmul(q2[:], V[l], child_view(0, NBLK, 2, l),
                                 start=(l == 0), stop=(l == 3))
            nc.vector.tensor_tensor(SA[:, :, 5:21], Db2[:], q2[:], ALU.mult)
            nc.vector.tensor_reduce(outp[:, OC_SA:OC_SA + NBLK],
                                    SA[:], axis=AX.X, op=ALU.add)
            for b0 in range(0, NBLK, 8):
                m2 = ps.tile([128, 4, 8, 16], f32, tag="m")
                for l in range(4):
                    nc.tensor.matmul(m2[:, l, :, :], W[l], child_view(b0, 8, 2, l),
                                     start=True, stop=True)
                nc.vector.tensor_tensor(
                    Dv[:, b0:b0 + 8, 24:88].rearrange("p b (n l) -> p b n l", l=4),
                    m2[:].transpose([0, 2, 3, 1]),
                    Db2[:, b0:b0 + 8, :, None].to_broadcast([128, 8, 16, 4]),
                    ALU.mult)
                # per-chunk endgame: S_b product + reductions
                nc.vector.tensor_tensor(sbv[:, b0:b0 + 8, 4:88],
                                        Dv[:, b0:b0 + 8, 4:88],
                                        lgv[:, b0:b0 + 8, 4:88], ALU.mult)
                nc.vector.tensor_reduce(outp[:, OC_SB + b0:OC_SB + b0 + 8],
                                        sbv[:, b0:b0 + 8, 4:88],
                                        axis=AX.X, op=ALU.add)
                rhov = Dv[:, b0:b0 + 8, 4:88].rearrange("p b (n l) -> p b l n", l=4)
                nc.vector.tensor_reduce(
                    outp[:, OC_RHO + 4 * b0:OC_RHO + 4 * (b0 + 8)]
                    .rearrange("p (b l) -> p b l", l=4),
                    rhov, axis=AX.X, op=ALU.add)

            nc.sync.dma_start(out=o_out, in_=outp[:])

    nc.finalize()
    return nc


_NC_CACHE = {}


def _shard_inputs(t, a, b, pi, sp):
    tb, tbl, wv, cores, gids, host = _host_prep(t, a, b, pi, sp)
    in_maps = []
    for k in range(NCORES):
        in_maps.append({"ebd": cores[k][0], "lgbd": cores[k][1], "wv": wv})
    return in_maps, host


def kernel(t, t_limits, a, b, pi, sp):
    from concourse.bass_utils import run_bass_kernel_spmd
    if "nc" not in _NC_CACHE:
        _NC_CACHE["nc"] = build_bass()
    nc = _NC_CACHE["nc"]
    in_maps, host = _shard_inputs(t, a, b, pi, sp)
    res = run_bass_kernel_spmd(nc, in_maps, list(range(NCORES)))
    return _combine(res.results, host)


# revision 8
# speedup vs baseline: 8.9329x; 1.1543x over previous
"""Bass/Trainium2 kernel for nn_BottomUpHTMM (bottom-up hidden tree Markov model).

Tree: complete 4-ary, depth 7, 21845 nodes. G=16 models, C=8 states, 256 labels.

v2 design:
- Each core owns 32 independent depth-3 subtrees rooted at the 256 level-4
  nodes (85..340): 32*(1+4+16+64) = 2720 nodes/core. Partition dim = (g,c).
- The prior chain of the reference cancels algebraically
  (prior*beta_il == A@beta_children), so only beta is propagated.
- Downward pass is factorized: eps(n) = eps(root_b) * D(n) where
  D(child) = D(parent)*bnr(parent)*m(child), bnr = 1/(A@beta_ch),
  m(child) = (W_l @ beta)(child). D is independent of anything above the
  subtree root, so each core emits per-root partial sums
  S_b = sum D*logb, S_A = sum Db*q, S_rho[l] = sum_{slot l} D, plus the root
  betas. No collective: the host computes the 341-node tree top and contracts
  eps(root) with the S terms.
- Emissions come from one ap_gather per buffer out of a host-built 1280-entry
  table (cols 0..255 = sm_b; cols 256+256*pos+label = leaf beta pre-normalized
  with sm_pi folded in). logb likewise (leaf entries include log pi).
- Reciprocals are computed as Exp(-Ln(x)) on the Scalar engine (DVE reciprocal
  is ~8 cycles/elem).

Per-core column layout: 32 blocks * 88 cols; block = [3 pad | root | 4 L1 |
16 L2 | 64 leaves]. Children of block-col c level j at co[j+1]+4*p+l,
co = [3, 4, 8, 24].
"""

import numpy as np
import ml_dtypes

BF16 = ml_dtypes.bfloat16

G = 16
C = 8
M = 256
L = 4
NCORES = 8
T_SIZE = 21845
NBLK = 32                 # subtrees per core
BLK = 88                  # cols per block (3 pad + 85 nodes)
NCOL = NBLK * BLK         # 2816
CO = [3, 4, 8, 24]        # level col offsets within block
O21 = [0, 1, 5]           # level offsets within the 21 internal slots
NB = [1, 4, 16]           # parents per block per level
NIDX = NCOL // 16         # 176

# out cols
OC_RB = 0     # 0..32   root beta
OC_SB = 32    # 32..64  S_b
OC_SA = 64    # 64..96  S_A
OC_RHO = 96   # 96..224 S_rho (block-major x l)
NOUT = 224


def _softmax(x, axis):
    e = np.exp(x - x.max(axis=axis, keepdims=True))
    return e / e.sum(axis=axis, keepdims=True)


def _wrap_idx(idx):
    """idx j at partition j%16, slot j//16, replicated across 8 gpsimd cores."""
    idx = np.asarray(idx, dtype=np.int16)
    n = len(idx)
    assert n % 16 == 0
    grid = idx.reshape(n // 16, 16).T          # [16, n/16]
    return np.tile(grid, (8, 1))               # [128, n/16]


def _block_cols():
    """Per-block arrays: node id builder + (col -> level, pos-slot)."""
    # block-relative heap ids per col (0 for pads), using cols 3..88
    rel = np.full(BLK, -1, np.int64)
    rel[3] = 0
    for p in range(21):                        # internal block-rel heap ids 0..20
        for l in range(4):
            rel[CO[1] + 4 * p + l if p == 0 else 0] = 0
    # simpler: levels explicitly
    rel[3] = 0
    rel[4:8] = np.arange(1, 5)
    rel[8:24] = np.arange(5, 21)
    rel[24:88] = np.arange(21, 85)
    return rel


_REL = _block_cols()


def _host_prep(t, a, b, pi, sp):
    t = np.asarray(t)
    labels = t[:, 0].astype(np.int64)
    a = np.asarray(a, np.float64)
    b = np.asarray(b, np.float64)
    pi = np.asarray(pi, np.float64)
    sp = np.asarray(sp, np.float64)
    sm_a = _softmax(a, 1)
    sm_b = _softmax(b, 2)
    sm_pi = _softmax(pi, 1)
    sm_sp = _softmax(sp, 1)
    asp = sm_a * sm_sp[:, None, None, :]

    # tables [128, 1280]
    tb = np.zeros((128, 1280), np.float32)
    tbl = np.zeros((128, 1280), np.float32)
    sb128 = sm_b.reshape(128, M)
    tb[:, :M] = sb128
    tbl[:, :M] = np.log(sb128)
    for pos in range(4):
        v = sm_b * sm_pi[:, :, pos][:, :, None]          # [G,C,M]
        s = v.sum(1, keepdims=True)
        tb[:, M + M * pos:M + M * (pos + 1)] = (v / s).reshape(128, M)
        tbl[:, M + M * pos:M + M * (pos + 1)] = np.log(v).reshape(128, M)

    # weights [128, 1152]: W0..3 V0..3 mbd
    la = np.log(sm_a)
    wv = np.zeros((128, 1024), np.float32)
    for l in range(4):
        Wl = np.zeros((128, 128))
        Vl = np.zeros((128, 128))
        for g in range(G):
            Wl[g * C:(g + 1) * C, g * C:(g + 1) * C] = asp[g, :, :, l].T      # [j,i]
            Vl[g * C:(g + 1) * C, g * C:(g + 1) * C] = (asp * la)[g, :, :, l].T
        wv[:, 128 * l:128 * (l + 1)] = Wl
        wv[:, 512 + 128 * l:512 + 128 * (l + 1)] = Vl

    wv = wv.astype(BF16)

    # per-core node ids + gather idx
    # block-rel heap: node 0 root, children of p at 4p+1+l
    gid_rel = np.zeros(85, np.int64)
    cores = []
    gids = []
    for k in range(NCORES):
        idx = np.zeros(NCOL, np.int64)
        gid_all = np.zeros((NBLK, 85), np.int64)
        for bq in range(NBLK):
            root = 85 + NBLK * k + bq
            gid_rel[0] = root
            for p in range(21):
                for l in range(4):
                    gid_rel[4 * p + 1 + l] = 4 * gid_rel[p] + 1 + l
            gid_all[bq] = gid_rel
            base = BLK * bq
            lab = labels[gid_rel]
            idx[base + 3:base + 24] = lab[:21]                     # internal: sm_b
            pos = (gid_rel[21:] - 1) % 4
            idx[base + 24:base + 88] = M + M * pos + lab[21:]      # leaves
        ebd = tb[:, idx].astype(np.float32)
        lgbd = tbl[:, idx].astype(np.float32)
        for bq in range(NBLK):
            ebd[:, BLK * bq:BLK * bq + 3] = 1.0
            lgbd[:, BLK * bq:BLK * bq + 3] = 0.0
        cores.append((ebd.astype(BF16), lgbd.astype(BF16)))
        gids.append(gid_all)

    host = dict(labels=labels, asp=asp, sm_b=sm_b, sm_pi=sm_pi,
                log_a=la, log_b=np.log(sm_b), log_sp=np.log(sm_sp))
    return tb, tbl, wv, cores, gids, host


def _combine(results, host):
    """Host: 341-node tree top + contraction with per-core S terms."""
    labels = host["labels"]; asp = host["asp"]; sm_b = host["sm_b"]
    log_a = host["log_a"]; log_b = host["log_b"]; log_sp = host["log_sp"]

    beta = np.zeros((341, G, C))
    Ab = np.zeros((85, G, C))
    for k in range(NCORES):
        rb = np.asarray(results[k]["out"], np.float64)
        rbm = rb[:, OC_RB:OC_RB + NBLK].T.reshape(NBLK, G, C)
        beta[85 + NBLK * k:85 + NBLK * (k + 1)] = \
            rbm / rbm.sum(2, keepdims=True)
    for lev in range(3, -1, -1):
        s, e = (4 ** lev - 1) // 3, (4 ** (lev + 1) - 1) // 3
        ch = 4 * np.arange(s, e)[:, None] + np.arange(1, 5)[None, :]
        AbP = np.einsum('gijl,plgj->pgi', asp, beta[ch])
        tmp = np.einsum('gcp,pgc->pgc', sm_b[:, :, labels[s:e]], AbP)
        beta[s:e] = tmp / tmp.sum(2, keepdims=True)
        Ab[s:e] = AbP
    eps = np.zeros((341, G, C)); eps[0] = beta[0]
    a_lh = np.zeros(G); rho = np.zeros((G, L))
    for lev in range(0, 4):
        s, e = (4 ** lev - 1) // 3, (4 ** (lev + 1) - 1) // 3
        ch = 4 * np.arange(s, e)[:, None] + np.arange(1, 5)[None, :]
        pe = eps[s:e] / Ab[s:e]
        mch = np.einsum('gijl,plgj->pgil', asp, beta[ch])
        epsc = pe[:, :, :, None] * mch
        for l in range(4):
            eps[ch[:, l]] = epsc[:, :, :, l]
        rho += epsc.sum(2).sum(0)
        a_lh += np.einsum('pgi,gijl,gijl,plgj->g', pe, asp, log_a, beta[ch])
    b_lh = np.einsum('ugc,gcu->g', eps, log_b[:, :, labels[:341]])

    # device terms
    for k in range(NCORES):
        out = np.asarray(results[k]["out"], np.float64)
        er = eps[85 + NBLK * k:85 + NBLK * (k + 1)].reshape(NBLK, 128)  # [b,(g,c)]
        S_b = out[:, OC_SB:OC_SB + NBLK].T          # [b, 128]
        S_A = out[:, OC_SA:OC_SA + NBLK].T
        S_r = out[:, OC_RHO:OC_RHO + 4 * NBLK].T.reshape(NBLK, 4, 128)
        b_lh += (er * S_b).reshape(NBLK, G, C).sum(0).sum(1)
        a_lh += (er * S_A).reshape(NBLK, G, C).sum(0).sum(1)
        rho += np.einsum('blp,bp->pl', S_r, er).reshape(G, C, L).sum(1)
    sp_lh = (rho * log_sp).sum(1)
    return (a_lh + b_lh + sp_lh).astype(np.float32)


def build_bass():
    import concourse.bacc as bacc
    import concourse.tile as tile
    import concourse.mybir as mybir
    from concourse import bass

    f32 = mybir.dt.float32
    bf16 = mybir.dt.bfloat16
    AF = mybir.ActivationFunctionType
    ALU = mybir.AluOpType
    AX = mybir.AxisListType

    nc = bacc.Bacc("TRN2", target_bir_lowering=False, debug=False,
                   num_devices=NCORES)

    eb_in = nc.dram_tensor("ebd", [128, NCOL], bf16, kind="ExternalInput").ap()
    lgb_in = nc.dram_tensor("lgbd", [128, NCOL], bf16, kind="ExternalInput").ap()
    wv_in = nc.dram_tensor("wv", [128, 1024], bf16, kind="ExternalInput").ap()
    o_out = nc.dram_tensor("out", [128, NOUT], f32, kind="ExternalOutput").ap()

    with tile.TileContext(nc) as tc:
        with tc.tile_pool(name="per", bufs=1) as per, \
             tc.tile_pool(name="wrk", bufs=2) as wrk, \
             tc.tile_pool(name="ps", bufs=2, space="PSUM") as ps, \
             tc.tile_pool(name="ps3", bufs=3, space="PSUM") as ps3:

            wv = per.tile([128, 1024], bf16, tag="wv")
            W = [wv[:, 128 * l:128 * (l + 1)] for l in range(4)]
            V = [wv[:, 512 + 128 * l:512 + 128 * (l + 1)] for l in range(4)]

            eb = per.tile([128, NCOL], bf16, tag="eb")      # emission -> beta
            lgb = per.tile([128, NCOL], bf16, tag="lgb")    # log emission
            nc.sync.dma_start(out=wv[:], in_=wv_in)
            nc.sync.dma_start(out=eb[:, :NCOL // 2], in_=eb_in[:, :NCOL // 2])
            nc.sync.dma_start(out=eb[:, NCOL // 2:], in_=eb_in[:, NCOL // 2:])
            nc.sync.dma_start(out=lgb[:], in_=lgb_in)
            Dt = per.tile([128, NCOL], bf16, tag="Dt")      # eps factor D
            sbp = per.tile([128, NCOL], bf16, tag="sbp")    # D*logb scratch
            bnr = per.tile([128, NBLK, 21], bf16, tag="bnr")
            SA = per.tile([128, NBLK, 21], f32, tag="SA")
            outp = per.tile([128, NOUT], f32, tag="outp")

            ebv = eb[:].rearrange("p (b c) -> p b c", b=NBLK)
            lgv = lgb[:].rearrange("p (b c) -> p b c", b=NBLK)
            Dv = Dt[:].rearrange("p (b c) -> p b c", b=NBLK)
            sbv = sbp[:].rearrange("p (b c) -> p b c", b=NBLK)

            def child_view(b0, nbl, j, l):
                """beta of l-th children of level-j parents: [128, nbl, NB[j]]"""
                v = ebv[:, b0:b0 + nbl, CO[j + 1]:CO[j + 1] + 4 * NB[j]]
                return v.rearrange("p b (n l) -> p b n l", l=4)[:, :, :, l]

            # ---------------- upward (no per-node normalization) ----------------
            for u, (j, b0, nbl) in enumerate([(2, 0, 16), (2, 16, 16),
                                              (1, 0, 32), (0, 0, 32)]):
                n_b = NB[j]
                ub = ps3.tile([128, nbl, n_b], f32, tag="ub")
                for l in range(4):
                    nc.tensor.matmul(ub[:], W[l], child_view(b0, nbl, j, l),
                                     start=(l == 0), stop=(l == 3))
                with nc.allow_low_precision(reason="bnr bf16 ok (tol 2e-2)"):
                    nc.vector.reciprocal(
                        bnr[:, b0:b0 + nbl, O21[j]:O21[j] + n_b], ub[:])
                nc.vector.tensor_tensor(
                    ebv[:, b0:b0 + nbl, CO[j]:CO[j] + n_b],
                    ebv[:, b0:b0 + nbl, CO[j]:CO[j] + n_b], ub[:], ALU.mult)

            # root (unnormalized) betas to output
            nc.scalar.copy(out=outp[:, OC_RB:OC_RB + NBLK], in_=ebv[:, :, 3])

            # ---------------- downward (D chain + S terms) ----------------
            Db0 = bnr[:, :, 0]                                   # [128, 32]
            # j = 0
            m0 = ps.tile([128, 4, NBLK], f32, tag="m")
            for l in range(4):
                nc.tensor.matmul(m0[:, l, :], W[l],
                                 child_view(0, NBLK, 0, l)
                                 .rearrange("p b n -> p (b n)"),
                                 start=True, stop=True)
            q0 = ps.tile([128, NBLK], f32, tag="q")
            for l in range(4):
                nc.tensor.matmul(q0[:], V[l],
                                 child_view(0, NBLK, 0, l)
                                 .rearrange("p b n -> p (b n)"),
                                 start=(l == 0), stop=(l == 3))
            nc.vector.tensor_tensor(Dv[:, :, 4:8], m0[:].transpose([0, 2, 1]),
                                    Db0[:, :, None].to_broadcast([128, NBLK, 4]),
                                    ALU.mult)
            nc.vector.tensor_tensor(SA[:, :, 0:1], Db0[:, :, None], q0[:, :, None],
                                    ALU.mult)
            # j = 1
            Db1 = wrk.tile([128, NBLK, 4], bf16, tag="db1")
            nc.vector.tensor_tensor(Db1[:], Dv[:, :, 4:8], bnr[:, :, 1:5], ALU.mult)
            m1 = ps.tile([128, 4, NBLK, 4], f32, tag="m")
            for l in range(4):
                nc.tensor.matmul(m1[:, l, :, :], W[l], child_view(0, NBLK, 1, l),
                                 start=True, stop=True)
            q1 = ps.tile([128, NBLK, 4], f32, tag="q")
            for l in range(4):
                nc.tensor.matmul(q1[:], V[l], child_view(0, NBLK, 1, l),
                                 start=(l == 0), stop=(l == 3))
            nc.vector.tensor_tensor(
                Dv[:, :, 8:24].rearrange("p b (n l) -> p b n l", l=4),
                m1[:].transpose([0, 2, 3, 1]),
                Db1[:, :, :, None].to_broadcast([128, NBLK, 4, 4]), ALU.mult)
            nc.vector.tensor_tensor(SA[:, :, 1:5], Db1[:], q1[:], ALU.mult)
            # j = 2
            Db2 = wrk.tile([128, NBLK, 16], bf16, tag="db2")
            nc.vector.tensor_tensor(Db2[:], Dv[:, :, 8:24], bnr[:, :, 5:21],
                                    ALU.mult)
            q2 = ps.tile([128, NBLK, 16], f32, tag="q")
            for l in range(4):
                nc.tensor.matmul(q2[:], V[l], child_view(0, NBLK, 2, l),
                                 start=(l == 0), stop=(l == 3))
            nc.vector.tensor_tensor(SA[:, :, 5:21], Db2[:], q2[:], ALU.mult)
            nc.vector.tensor_reduce(outp[:, OC_SA:OC_SA + NBLK],
                                    SA[:], axis=AX.X, op=ALU.add)
            for b0 in range(0, NBLK, 8):
                m2 = ps.tile([128, 4, 8, 16], f32, tag="m")
                for l in range(4):
                    nc.tensor.matmul(m2[:, l, :, :], W[l], child_view(b0, 8, 2, l),
                                     start=True, stop=True)
                nc.vector.tensor_tensor(
                    Dv[:, b0:b0 + 8, 24:88].rearrange("p b (n l) -> p b n l", l=4),
                    m2[:].transpose([0, 2, 3, 1]),
                    Db2[:, b0:b0 + 8, :, None].to_broadcast([128, 8, 16, 4]),
                    ALU.mult)
                # per-chunk endgame: S_b product + reductions
                nc.vector.tensor_tensor(sbv[:, b0:b0 + 8, 4:88],
                                        Dv[:, b0:b0 + 8, 4:88],
                                        lgv[:, b0:b0 + 8, 4:88], ALU.mult)
                nc.vector.tensor_reduce(outp[:, OC_SB + b0:OC_SB + b0 + 8],
                                        sbv[:, b0:b0 + 8, 4:88],
                                        axis=AX.X, op=ALU.add)
                rhov = Dv[:, b0:b0 + 8, 4:88].rearrange("p b (n l) -> p b l n", l=4)
                nc.vector.tensor_reduce(
                    outp[:, OC_RHO + 4 * b0:OC_RHO + 4 * (b0 + 8)]
                    .rearrange("p (b l) -> p b l", l=4),
                    rhov, axis=AX.X, op=ALU.add)

            nc.sync.dma_start(out=o_out, in_=outp[:])

    nc.finalize()
    return nc


_NC_CACHE = {}


def _shard_inputs(t, a, b, pi, sp):
    tb, tbl, wv, cores, gids, host = _host_prep(t, a, b, pi, sp)
    in_maps = []
    for k in range(NCORES):
        in_maps.append({"ebd": cores[k][0], "lgbd": cores[k][1], "wv": wv})
    return in_maps, host


def kernel(t, t_limits, a, b, pi, sp):
    from concourse.bass_utils import run_bass_kernel_spmd
    if "nc" not in _NC_CACHE:
        _NC_CACHE["nc"] = build_bass()
    nc = _NC_CACHE["nc"]
    in_maps, host = _shard_inputs(t, a, b, pi, sp)
    res = run_bass_kernel_spmd(nc, in_maps, list(range(NCORES)))
    return _combine(res.results, host)
